# revision 27
# baseline (speedup 1.0000x reference)
"""MLGRU cell on 8 Trainium2 NeuronCores.

Reference math (per batch element b, all matmuls contract over d=2048):
    f = sigmoid(x @ tern(wf).T + bf)
    c = silu   (x @ tern(wc).T + bc)
    h = f * h_prev + (1 - f) * c
    g = sigmoid(x @ tern(wg).T + bg)
    o = (g * h) @ tern(wo).T + bo
    return (o, h)

Sharding: data-parallel over batch, one batch element per core (B=8, 8 cores,
no collectives).

Precision/speed trick: ternary weights are EXACT in fp8e4 (TRN E4M3, max 240).
Activations are split hi/lo: v = fp8(v) + fp8(v - fp8(v)); each 256-deep
contraction chunk runs as a DoubleRow fp8 matmul pair (hi, lo) sharing the
same stationary weight tile. DoubleRow processes 2 contraction rows per cell
per cycle, so the pair costs ~the same as a single bf16 matmul of half the
contraction depth -> ~2x PE throughput vs bf16, with BETTER accuracy than
bf16 (measured ~3e-3 rel vs 6e-3).

Device layout: features-on-partitions ([o, t] tiles) everywhere, so neither
activations nor weights ever need an on-device transpose:
  - stage A matmul: lhsT = w_t[d_pair_chunk, o_chunk]  (K=d on partitions),
                    rhs  = xhi/xlo[d_pair_chunk, t_chunk]  -> psum [o, t]
  - gating is elementwise in [o, t]; u = g*h is split hi/lo into the
    [feature, t] fp8 layout that stage B needs as its rhs (K = feature dim).
Weights are ternarized + transposed + cast fp8 host-side; x is transposed +
hi/lo split host-side. h_prev ships bf16. PSUM accumulation is fp32.

Error-budget spend: the lo-residual matmul of pair-chunk 0 is dropped for the
c projection and for stage B (128 of 4096 matmuls). Full-input simulation:
o rel 1.37e-2, h rel 9.7e-3 vs the 2e-2 gate (1.46x margin); hardware has
matched the simulation to ~3 digits throughout.
"""

import sys

if "/opt/trn_rl_repo" not in sys.path:
    sys.path.insert(0, "/opt/trn_rl_repo")

import numpy as np
import ml_dtypes

import concourse.bass as bass
import concourse.mybir as mybir
import concourse.tile as tile
from concourse import bacc
from concourse.bass_utils import run_bass_kernel_spmd

BF16 = ml_dtypes.bfloat16
FP8 = ml_dtypes.float8_e4m3   # TRN fp8e4: max +-240, matches ml_dtypes e4m3
B, S, D = 8, 2048, 2048
P = 128
KO = D // P    # 16 contraction chunks of 128
NC2 = KO // 2  # 8 DoubleRow pair-chunks of 256
NJ = D // P    # 16 output-feature blocks
TB = 512       # token-block (matmul free dim / PSUM bank)
NTB = S // TB  # 4
THRESH = np.float32(0.33)

F32 = mybir.dt.float32
BF = mybir.dt.bfloat16
F8 = mybir.dt.float8e4
AF = mybir.ActivationFunctionType
DR = mybir.MatmulPerfMode.DoubleRow

_CACHE = {}


def build_nc(nrep=1):
    """Per-core Bass program. Inputs are pre-formatted host-side (see kernel).

    nrep > 1 repeats the whole computation back-to-back (bench-only: lets a
    single NEFF execution amortize host dispatch overhead so steady-state
    per-iteration device time is measurable through a noisy dispatch path).
    """
    nc = bacc.Bacc("TRN2", target_bir_lowering=False, debug=False, num_devices=8)

    xhi_d = nc.dram_tensor("xhi", (P, KO, S), F8, kind="ExternalInput")
    xlo_d = nc.dram_tensor("xlo", (P, KO, S), F8, kind="ExternalInput")
    hpt_d = nc.dram_tensor("hpt", (D, S), BF, kind="ExternalInput")
    w_d = {
        k: nc.dram_tensor(f"w4{k}", (NJ, P, KO, P), F8, kind="ExternalInput")
        for k in ("f", "c", "g", "o")
    }
    b_d = {
        k: nc.dram_tensor(f"b4{k}", (P, NJ), F32, kind="ExternalInput")
        for k in ("f", "c", "g", "o")
    }
    ht_d = nc.dram_tensor("ht", (D, S), F32, kind="ExternalOutput")
    ot_d = nc.dram_tensor("ot", (D, S), F32, kind="ExternalOutput")

    with tile.TileContext(nc) as tc:
        with (
            tc.tile_pool(name="xp", bufs=1) as xp,
            tc.tile_pool(name="up", bufs=1) as up,
            tc.tile_pool(name="wp", bufs=8) as wp,
            tc.tile_pool(name="wop", bufs=4) as wop,
            tc.tile_pool(name="hpp", bufs=3) as hpp,
            tc.tile_pool(name="actp", bufs=3) as actp,
            tc.tile_pool(name="tmpp", bufs=3) as tmpp,
            tc.tile_pool(name="outp", bufs=3) as outp,
            tc.tile_pool(name="biasp", bufs=1) as biasp,
            tc.tile_pool(name="psum", bufs=8, space="PSUM") as psum,
        ):
          for _rep in range(nrep):
            # first x block + first gate weights up front so the PE can
            # start early instead of waiting on all of x + weights
            # startup-critical DMA order: the first chain (gate f, tb 0,
            # chunk 0) needs only w0f + x quarter 0 (~0.5MB) -- interleave
            # weights and x quarters so it can start as early as possible
            w0 = {}
            NQ = 4            # tb=0 in quarter tiles: PE starts after ~0.5MB
            KQ = KO // NQ
            q_hi, q_lo = [], []
            wf0 = wp.tile([P, KO, P], F8, tag="wgate", name="w0f")
            nc.sync.dma_start(wf0[:], w_d["f"][0])
            w0["f"] = wf0
            for qi in range(NQ):
                ksl = slice(qi * KQ, (qi + 1) * KQ)
                t = xp.tile([P, KQ, TB], F8, tag=f"xhi0q{qi}", name=f"xhi0q{qi}")
                nc.sync.dma_start(t[:], xhi_d[:, ksl, 0:TB])
                q_hi.append(t)
                t = xp.tile([P, KQ, TB], F8, tag=f"xlo0q{qi}", name=f"xlo0q{qi}")
                nc.sync.dma_start(t[:], xlo_d[:, ksl, 0:TB])
                q_lo.append(t)
                if qi == 0:
                    w = wp.tile([P, KO, P], F8, tag="wgate", name="w0c")
                    nc.sync.dma_start(w[:], w_d["c"][0])
                    w0["c"] = w
                elif qi == 1:
                    w = wp.tile([P, KO, P], F8, tag="wgate", name="w0g")
                    nc.sync.dma_start(w[:], w_d["g"][0])
                    w0["g"] = w
            xhis, xlos = [], []
            xhis.append(tuple(q_hi))
            xlos.append(tuple(q_lo))
            for tb in range(1, NTB):
                t = xp.tile([P, KO, TB], F8, tag=f"xhi{tb}", name=f"xhi{tb}")
                nc.sync.dma_start(t[:], xhi_d[:, :, tb * TB:(tb + 1) * TB])
                xhis.append(t)
                t = xp.tile([P, KO, TB], F8, tag=f"xlo{tb}", name=f"xlo{tb}")
                nc.sync.dma_start(t[:], xlo_d[:, :, tb * TB:(tb + 1) * TB])
                xlos.append(t)
            bt = {}
            for k in ("f", "c", "g", "o"):
                t = biasp.tile([P, NJ], F32, tag=f"bias_{k}")
                nc.sync.dma_start(t[:], b_d[k][:])
                bt[k] = t
            uhis = [up.tile([P, KO, TB], F8, tag=f"uhi{tb}", name=f"uhi{tb}")
                    for tb in range(NTB)]
            ulos = [up.tile([P, KO, TB], F8, tag=f"ulo{tb}", name=f"ulo{tb}")
                    for tb in range(NTB)]

            def dr_chains(pss, w, rhis, rlos, tbs, tb_major=False,
                          skip_lo=()):
                # chunk-major: one stationary load feeds len(tbs) x (hi, lo)
                # DoubleRow matmuls (LDWEIGHTS dedup). tb_major instead
                # finishes one token block at a time (warm-up: fewer x tiles
                # needed before the PE can start). skip_lo: pair-chunks whose
                # lo-residual matmul is dropped (spends a slice of the 2e-2
                # error budget; see kernel() docstring).
                outer = tbs if tb_major else range(NC2)
                inner = range(NC2) if tb_major else tbs
                def rat(rs, tb, c):
                    t = rs[tb]
                    if isinstance(t, tuple):
                        per = NC2 // len(t)     # pair-chunks per sub-tile
                        tt = t[c // per]
                        cc = c % per
                        return tt[:, 2 * cc:2 * cc + 2, :]
                    return t[:, 2 * c:2 * c + 2, :]

                for a in outer:
                    for b in inner:
                        c, tb, i = (b, a, tbs.index(a)) if tb_major else (a, b, tbs.index(b))
                        ks = slice(2 * c, 2 * c + 2)
                        nc.tensor.matmul(pss[i][:], w[:, ks, :],
                                         rat(rhis, tb, c),
                                         start=(c == 0),
                                         stop=(c == NC2 - 1 and
                                               NC2 - 1 in skip_lo),
                                         perf_mode=DR, skip_group_check=True)
                        if c not in skip_lo:
                            nc.tensor.matmul(pss[i][:], w[:, ks, :],
                                             rat(rlos, tb, c),
                                             start=False,
                                             stop=(c == NC2 - 1),
                                             perf_mode=DR,
                                             skip_group_check=True)

            # ---- stage A: f/c/g projections + gating, fills U ----
            for j in range(NJ):
                if j == 0:
                    wt = w0
                else:
                    wt = {}
                    for k in ("f", "c", "g"):
                        w = wp.tile([P, KO, P], F8, tag="wgate")
                        nc.sync.dma_start(w[:], w_d[k][j])
                        wt[k] = w
                ps = {}
                for k in ("f", "c", "g"):
                    pss = [psum.tile([P, TB], F32, tag="ps", name=f"psA{k}{tb}")
                           for tb in range(NTB)]
                    dr_chains(pss, wt[k], xhis, xlos, list(range(NTB)),
                              tb_major=(j == 0),
                              skip_lo=(0,) if k == "c" else ())
                    ps[k] = pss
                for tb in range(NTB):
                    ts_ = slice(tb * TB, (tb + 1) * TB)
                    fs = actp.tile([P, TB], BF, tag="fs")
                    nc.scalar.activation(fs[:], ps["f"][tb][:], AF.Sigmoid,
                                         bias=bt["f"][:, j:j + 1])
                    sg = actp.tile([P, TB], F32, tag="sg")
                    nc.scalar.activation(sg[:], ps["c"][tb][:], AF.Sigmoid,
                                         bias=bt["c"][:, j:j + 1])
                    c0b = tmpp.tile([P, TB], F32, tag="c0b")
                    nc.vector.tensor_scalar(c0b[:], ps["c"][tb][:],
                                            bt["c"][:, j:j + 1], None,
                                            mybir.AluOpType.add)
                    cs = actp.tile([P, TB], F32, tag="cs")
                    nc.gpsimd.tensor_mul(cs[:], c0b[:], sg[:])
                    gs = actp.tile([P, TB], BF, tag="gs")
                    nc.scalar.activation(gs[:], ps["g"][tb][:], AF.Sigmoid,
                                         bias=bt["g"][:, j:j + 1])
                    hp = hpp.tile([P, TB], BF, tag="hp")
                    nc.sync.dma_start(hp[:], hpt_d[j * P:(j + 1) * P, ts_])
                    # h = c + f*(h_prev - c)
                    d1 = tmpp.tile([P, TB], BF, tag="d1")
                    nc.vector.tensor_sub(d1[:], hp[:], cs[:])
                    d2 = tmpp.tile([P, TB], BF, tag="d2")
                    nc.vector.tensor_mul(d2[:], fs[:], d1[:])
                    hs = outp.tile([P, TB], F32, tag="hs")
                    nc.vector.tensor_add(hs[:], d2[:], cs[:])
                    nc.sync.dma_start(ht_d[j * P:(j + 1) * P, ts_], hs[:])
                    # u = g*h split hi/lo to fp8 for stage B
                    u32 = tmpp.tile([P, TB], F32, tag="u32")
                    nc.vector.tensor_mul(u32[:], gs[:], hs[:])
                    nc.scalar.copy(uhis[tb][:, j, :], u32[:])
                    nc.vector.tensor_sub(ulos[tb][:, j, :], u32[:],
                                         uhis[tb][:, j, :])

            # ---- stage B: o = U-contraction with wo ----
            # 4 PSUM banks per j, 8 matmuls share each LDWEIGHTS
            for j in range(NJ):
                w = wop.tile([P, KO, P], F8, tag="wo")
                nc.sync.dma_start(w[:], w_d["o"][j])
                pss = [psum.tile([P, TB], F32, tag="ps", name=f"psB{tb}")
                       for tb in range(NTB)]
                dr_chains(pss, w, uhis, ulos, list(range(NTB)),
                          skip_lo=(0,))
                for tb in range(NTB):
                    ts_ = slice(tb * TB, (tb + 1) * TB)
                    os_ = outp.tile([P, TB], F32, tag="os")
                    nc.vector.tensor_scalar(os_[:], pss[tb][:],
                                            bt["o"][:, j:j + 1],
                                            None, mybir.AluOpType.add)
                    nc.scalar.dma_start(ot_d[j * P:(j + 1) * P, ts_], os_[:])

    _dedupe_ldweights(nc)
    nc.compile()
    return nc


def _dedupe_ldweights(nc):
    """Remove InstLdweights identical to the previous one (only matmuls in
    between): the PE array keeps the stationary operand across matmuls, so a
    reload of the same weights is pure overhead (~LDW streams 256 columns in
    DoubleRow mode). Waits on a removed load migrate to the next instruction.
    """

    def key(ins):
        ap = ins.ins[0]
        return (ap.memref, ap.offset, str(ap.ap), str(ap.dtype),
                str(getattr(ins, "perf_mode", None)))

    removed = 0
    for fn in nc.m.functions:
        for blk in fn.blocks:
            out, last = [], None
            for ins in blk.instructions:
                nm = type(ins).__name__
                if nm == "InstLdweights":
                    k = key(ins)
                    if last is not None and k == last:
                        si = ins.sync_info
                        if si is None or (not si.on_wait and
                                          not si.on_update):
                            # drop only sync-free duplicates: removing an
                            # instruction that carries waits/updates would
                            # require migrating them, which risks reordering
                            # a handshake relative to interleaved semaphore
                            # instructions
                            removed += 1
                            continue
                        out.append(ins)
                        continue
                    last = k
                elif nm != "InstMatmult" and ins.engine == mybir.EngineType.PE:
                    last = None
                out.append(ins)
            blk.instructions = out
    return removed


def _ternary(w):
    return np.where(np.abs(w) < THRESH, np.float32(0.0),
                    np.sign(w)).astype(np.float32)


def _fmt_w(w):
    # w: [o, d] fp32 -> w4[j, kp, ko, oi] = tern(w)[j*128+oi, ko*128+kp], fp8
    wt = _ternary(np.asarray(w)).T  # [d, o]
    w4 = wt.reshape(KO, P, NJ, P).transpose(2, 1, 0, 3)
    return np.ascontiguousarray(w4).astype(FP8)


def _fmt_x(xb):
    # xb: [t, d] fp32 -> xt[kp, ko, t] = xb[t, ko*128+kp], split hi/lo fp8
    xt = np.asarray(xb).T.reshape(KO, P, S).transpose(1, 0, 2)
    xt = np.ascontiguousarray(xt)
    xhi = xt.astype(FP8)
    xlo = (xt - xhi.astype(np.float32)).astype(FP8)
    return xhi, xlo


def kernel(x, h_prev, wf, bf, wc, bc, wg, bg, wo, bo):
    if "nc" not in _CACHE:
        _CACHE["nc"] = build_nc()
    nc = _CACHE["nc"]

    w4 = {"f": _fmt_w(wf), "c": _fmt_w(wc), "g": _fmt_w(wg), "o": _fmt_w(wo)}
    b4 = {
        k: np.ascontiguousarray(np.asarray(v, np.float32).reshape(NJ, P).T)
        for k, v in (("f", bf), ("c", bc), ("g", bg), ("o", bo))
    }
    in_maps = []
    for b in range(B):
        xhi, xlo = _fmt_x(x[b])
        m = {"xhi": xhi, "xlo": xlo,
             "hpt": np.ascontiguousarray(np.asarray(h_prev[b]).T).astype(BF16)}
        for k in ("f", "c", "g", "o"):
            m[f"w4{k}"] = w4[k]
            m[f"b4{k}"] = b4[k]
        in_maps.append(m)

    res = run_bass_kernel_spmd(nc, in_maps, core_ids=list(range(B)))

    o = np.empty((B, S, D), np.float32)
    h = np.empty((B, S, D), np.float32)
    for b in range(B):
        o[b] = res.results[b]["ot"].T
        h[b] = res.results[b]["ht"].T
    return o, h
